# revision 10
# baseline (speedup 1.0000x reference)
"""Trainium2 Bass kernel for nn_Dist2CycleLayer.

Computes out = relu(adjacency * Linv) @ W.T + b  with N = 8192.
(x_e is an input of the nn.Module but is discarded by its forward pass,
so it is never shipped to the device.)

Sharding: row-partition the [N, N] matrices across 8 NeuronCores
(1024 rows per core). Each core computes its 1024 output rows fully
(the reduction over the 8192 columns is row-local); outputs are
concatenated on the host.

Per-core device program (row tile = 128 partitions, column chunk = 4096):
  DMA  a = adj[rt, ch] (SP HWDGE ring), l = linv[rt, ch] (ACT HWDGE ring)
  DVE  a <- a * l                       (tensor_tensor mult, in place)
  DVE  s = max(a, 0) * Wb ; acc[:, ch] = sum_j s   (scalar_tensor_tensor:
                                         fused relu + weight mult + row sum)
  per row tile: stage[:, rt] = b + sum(acc)  (tensor_tensor_reduce)
  one [128, 8] result DMA per core at the end (gpsimd/SWDGE ring).

W is broadcast once to all 128 partitions ([128, 8192] resident in SBUF)
via the SWDGE ring so the two HWDGE rings carry only the input streams.

The device output is [128, 8] with element [p, rt] = row rt*128+p; the
host de-interleaves with .T.reshape(-1, 1).
"""

import numpy as np

import os

N = 8192
N_CORES = 8
ROWS = N // N_CORES  # 1024 rows per core
P = 128  # partitions
CHUNK = int(os.environ.get("K_CHUNK", "4096"))
N_CHUNKS = N // CHUNK
N_RTILES = ROWS // P
IO_BUFS = int(os.environ.get("K_IO_BUFS", "3"))

_CACHE = {}


def _build(reps=1):
    import concourse.bacc as bacc
    import concourse.mybir as mybir
    from concourse import tile

    f32 = mybir.dt.float32
    Alu = mybir.AluOpType

    nc = bacc.Bacc(
        "TRN2",
        target_bir_lowering=False,
        debug=False,
        num_devices=N_CORES,
    )

    adj = nc.dram_tensor("adj", [ROWS, N], f32, kind="ExternalInput").ap()
    linv = nc.dram_tensor("linv", [ROWS, N], f32, kind="ExternalInput").ap()
    w = nc.dram_tensor("w", [1, N], f32, kind="ExternalInput").ap()
    b = nc.dram_tensor("b", [1, 1], f32, kind="ExternalInput").ap()
    out = nc.dram_tensor("out", [P, N_RTILES], f32, kind="ExternalOutput").ap()

    with tile.TileContext(nc) as tc:
        with (
            tc.tile_pool(name="consts", bufs=1) as consts,
            tc.tile_pool(name="io", bufs=IO_BUFS) as io,
            tc.tile_pool(name="sink", bufs=1) as sink,
            tc.tile_pool(name="small", bufs=2) as small,
        ):
            # W broadcast to all partitions, resident for the whole kernel.
            # ACT HWDGE ring (SWDGE stride-0 broadcast hangs the device).
            wb = consts.tile([P, N], f32)
            nc.scalar.dma_start(out=wb[:], in_=w.broadcast_to([P, N]))
            # b broadcast to all partitions.
            b_bc = consts.tile([P, 1], f32)
            nc.scalar.dma_start(out=b_bc[:], in_=b.broadcast_to([P, 1]))

            for rep in range(reps):
                stage = small.tile([P, N_RTILES], f32, tag="stage")
                for rt in range(N_RTILES):
                    r0 = rt * P
                    acc = small.tile([P, N_CHUNKS], f32, tag="acc")
                    for ch in range(N_CHUNKS):
                        c0 = ch * CHUNK
                        a_t = io.tile([P, CHUNK], f32, tag="a")
                        l_t = io.tile([P, CHUNK], f32, tag="l")
                        # Two input streams on the two HWDGE rings.
                        nc.sync.dma_start(
                            out=a_t[:], in_=adj[r0 : r0 + P, c0 : c0 + CHUNK]
                        )
                        nc.scalar.dma_start(
                            out=l_t[:], in_=linv[r0 : r0 + P, c0 : c0 + CHUNK]
                        )
                        # In-place product: a_t <- a_t * l_t (identical APs
                        # are safe on the DVE streaming pipe).
                        nc.vector.tensor_mul(out=a_t[:], in0=a_t[:], in1=l_t[:])
                        s = sink.tile([P, CHUNK], f32, tag="s")
                        nc.vector.scalar_tensor_tensor(
                            out=s[:],
                            in0=a_t[:],
                            scalar=0.0,
                            in1=wb[:, c0 : c0 + CHUNK],
                            op0=Alu.max,
                            op1=Alu.mult,
                            accum_out=acc[:, ch : ch + 1],
                        )
                    # stage[:, rt] = b + sum(acc)
                    res = small.tile([P, 1], f32, tag="res")
                    nc.vector.tensor_reduce(
                        out=res[:], in_=acc[:], axis=mybir.AxisListType.X, op=Alu.add
                    )
                    nc.vector.tensor_add(
                        out=stage[:, rt : rt + 1], in0=res[:], in1=b_bc[:]
                    )
                nc.sync.dma_start(out=out[:, :], in_=stage[:])

    nc.compile()
    return nc


def get_nc(reps=1):
    key = ("nc", reps)
    if key not in _CACHE:
        _CACHE[key] = _build(reps)
    return _CACHE[key]


def make_in_maps(adjacency, Linv, W, b):
    adjacency = np.ascontiguousarray(adjacency, dtype=np.float32)
    Linv = np.ascontiguousarray(Linv, dtype=np.float32)
    W = np.ascontiguousarray(W, dtype=np.float32).reshape(1, N)
    b = np.ascontiguousarray(b, dtype=np.float32).reshape(1, 1)
    in_maps = []
    for c in range(N_CORES):
        r0, r1 = c * ROWS, (c + 1) * ROWS
        in_maps.append(
            {
                "adj": adjacency[r0:r1],
                "linv": Linv[r0:r1],
                "w": W,
                "b": b,
            }
        )
    return in_maps


def unstage(core_out):
    """Device [128, 8] staging layout -> [1024, 1] rows for one core."""
    return np.ascontiguousarray(core_out.T).reshape(ROWS, 1)


def kernel(x_e=None, Linv=None, adjacency=None, W=None, b=None, **_unused):
    from concourse.bass_utils import run_bass_kernel_spmd

    nc = get_nc()
    in_maps = make_in_maps(adjacency, Linv, W, b)
    res = run_bass_kernel_spmd(nc, in_maps, core_ids=list(range(N_CORES)))
    out = np.concatenate([unstage(r["out"]) for r in res.results], axis=0)
    return out.astype(np.float32)


# revision 19
# speedup vs baseline: 1.1429x; 1.1429x over previous
"""Trainium2 Bass kernel for nn_Dist2CycleLayer.

Computes out = relu(adjacency * Linv) @ W.T + b  with N = 8192.
(x_e is an input of the nn.Module but is discarded by its forward pass,
so it is never shipped to the device.)

Sharding: row-partition the [N, N] matrices across 8 NeuronCores
(1024 rows per core). Each core computes its 1024 output rows fully
(the reduction over the 8192 columns is row-local); outputs are
concatenated on the host.

Per-core device program (row tile = 128 partitions, column chunk = 4096):
  DMA  a = adj[rt, ch] (SP HWDGE ring), l = linv[rt, ch] (ACT HWDGE ring)
  DVE  a <- a * l                       (tensor_tensor mult, in place)
  DVE  s = max(a, 0) * Wb ; acc[:, ch] = sum_j s   (scalar_tensor_tensor:
                                         fused relu + weight mult + row sum)
  per row tile: stage[:, rt] = reduce_add(acc) + b
  one [128, 8] result DMA per core at the end.

W is broadcast once to all 128 partitions ([128, 8192] resident in SBUF,
stride-0 source DMA on the ACT HWDGE ring). Results are staged in a
single [128, 8] tile so no tiny per-row-tile DMAs pollute the input
rings (element [p, rt] = output row rt*128+p; the host de-interleaves
with .T.reshape(-1, 1)).

Measured on the axon-tunneled trn2 cores: ~205-235 us device body time
(HBM roofline for the 64 MiB/core input stream at ~358 GB/s is ~187 us);
DVE busy ~142 us is fully hidden. Relative error vs the fp32 jax
reference: ~8.5e-07.
"""

import numpy as np

import os

N = 8192
N_CORES = 8
ROWS = N // N_CORES  # 1024 rows per core
P = 128  # partitions
CHUNK = int(os.environ.get("K_CHUNK", "4096"))
N_CHUNKS = N // CHUNK
N_RTILES = ROWS // P
IO_BUFS = int(os.environ.get("K_IO_BUFS", "3"))

_CACHE = {}


def _build(reps=1):
    import concourse.bacc as bacc
    import concourse.mybir as mybir
    from concourse import tile

    f32 = mybir.dt.float32
    Alu = mybir.AluOpType

    nc = bacc.Bacc(
        "TRN2",
        target_bir_lowering=False,
        debug=False,
        num_devices=N_CORES,
    )

    adj = nc.dram_tensor("adj", [ROWS, N], f32, kind="ExternalInput").ap()
    linv = nc.dram_tensor("linv", [ROWS, N], f32, kind="ExternalInput").ap()
    w = nc.dram_tensor("w", [1, N], f32, kind="ExternalInput").ap()
    b = nc.dram_tensor("b", [1, 1], f32, kind="ExternalInput").ap()
    out = nc.dram_tensor("out", [P, N_RTILES], f32, kind="ExternalOutput").ap()

    with tile.TileContext(nc) as tc:
        with (
            tc.tile_pool(name="consts", bufs=1) as consts,
            tc.tile_pool(name="io", bufs=IO_BUFS) as io,
            tc.tile_pool(name="sink", bufs=1) as sink,
            tc.tile_pool(name="small", bufs=2) as small,
        ):
            # W broadcast to all partitions, resident for the whole kernel.
            # ACT HWDGE ring (SWDGE stride-0 broadcast hangs the device).
            wb = consts.tile([P, N], f32)
            nc.scalar.dma_start(out=wb[:], in_=w.broadcast_to([P, N]))
            # b broadcast to all partitions.
            b_bc = consts.tile([P, 1], f32)
            nc.scalar.dma_start(out=b_bc[:], in_=b.broadcast_to([P, 1]))

            for rep in range(reps):
                stage = small.tile([P, N_RTILES], f32, tag="stage")
                for rt in range(N_RTILES):
                    r0 = rt * P
                    acc = small.tile([P, N_CHUNKS], f32, tag="acc")
                    for ch in range(N_CHUNKS):
                        c0 = ch * CHUNK
                        a_t = io.tile([P, CHUNK], f32, tag="a")
                        l_t = io.tile([P, CHUNK], f32, tag="l")
                        # Two input streams on the two HWDGE rings.
                        nc.sync.dma_start(
                            out=a_t[:], in_=adj[r0 : r0 + P, c0 : c0 + CHUNK]
                        )
                        nc.scalar.dma_start(
                            out=l_t[:], in_=linv[r0 : r0 + P, c0 : c0 + CHUNK]
                        )
                        # In-place product: a_t <- a_t * l_t (identical APs
                        # are safe on the DVE streaming pipe).
                        nc.vector.tensor_mul(out=a_t[:], in0=a_t[:], in1=l_t[:])
                        s = sink.tile([P, CHUNK], f32, tag="s")
                        nc.vector.scalar_tensor_tensor(
                            out=s[:],
                            in0=a_t[:],
                            scalar=0.0,
                            in1=wb[:, c0 : c0 + CHUNK],
                            op0=Alu.max,
                            op1=Alu.mult,
                            accum_out=acc[:, ch : ch + 1],
                        )
                    # stage[:, rt] = b + sum(acc)
                    res = small.tile([P, 1], f32, tag="res")
                    nc.vector.tensor_reduce(
                        out=res[:], in_=acc[:], axis=mybir.AxisListType.X, op=Alu.add
                    )
                    nc.vector.tensor_add(
                        out=stage[:, rt : rt + 1], in0=res[:], in1=b_bc[:]
                    )
                nc.sync.dma_start(out=out[:, :], in_=stage[:])

    nc.compile()
    return nc


def get_nc(reps=1):
    key = ("nc", reps)
    if key not in _CACHE:
        _CACHE[key] = _build(reps)
    return _CACHE[key]


def make_in_maps(adjacency, Linv, W, b):
    adjacency = np.ascontiguousarray(adjacency, dtype=np.float32)
    Linv = np.ascontiguousarray(Linv, dtype=np.float32)
    W = np.ascontiguousarray(W, dtype=np.float32).reshape(1, N)
    b = np.ascontiguousarray(b, dtype=np.float32).reshape(1, 1)
    in_maps = []
    for c in range(N_CORES):
        r0, r1 = c * ROWS, (c + 1) * ROWS
        in_maps.append(
            {
                "adj": adjacency[r0:r1],
                "linv": Linv[r0:r1],
                "w": W,
                "b": b,
            }
        )
    return in_maps


def unstage(core_out, b=0.0):
    """Device staging layout -> [1024, 1] output rows for one core.

    [128, 8], element [p, rt] = row rt*128 + p (b already added on
    device).
    """
    return np.ascontiguousarray(core_out.T).reshape(ROWS, 1)


def kernel(x_e=None, Linv=None, adjacency=None, W=None, b=None, **_unused):
    from concourse.bass_utils import run_bass_kernel_spmd

    nc = get_nc()
    in_maps = make_in_maps(adjacency, Linv, W, b)
    res = run_bass_kernel_spmd(nc, in_maps, core_ids=list(range(N_CORES)))
    out = np.concatenate([unstage(r["out"], b) for r in res.results], axis=0)
    return out.astype(np.float32)


# revision 21
# speedup vs baseline: 1.1599x; 1.0149x over previous
"""Trainium2 Bass kernel for nn_Dist2CycleLayer.

Computes out = relu(adjacency * Linv) @ W.T + b  with N = 8192.
(x_e is an input of the nn.Module but is discarded by its forward pass,
so it is never shipped to the device.)

Sharding: row-partition the [N, N] matrices across 8 NeuronCores
(1024 rows per core). Each core computes its 1024 output rows fully
(the reduction over the 8192 columns is row-local); outputs are
concatenated on the host.

Per-core device program (row tile = 128 partitions, column chunk = 4096):
  DMA  a = adj[rt, ch] (SP HWDGE ring), l = linv[rt, ch] (ACT HWDGE ring)
  DVE  a <- a * l                       (tensor_tensor mult, in place)
  DVE  s = max(a, 0) * Wb ; acc[:, ch] = sum_j s   (scalar_tensor_tensor:
                                         fused relu + weight mult + row sum)
  per row tile: stage[:, rt] = reduce_add(acc) + b
  one [128, 8] result DMA per core at the end.

W is broadcast once to all 128 partitions ([128, 8192] resident in SBUF,
stride-0 source DMA on the ACT HWDGE ring). Results are staged in a
single [128, 8] tile so no tiny per-row-tile DMAs pollute the input
rings (element [p, rt] = output row rt*128+p; the host de-interleaves
with .T.reshape(-1, 1)).

Measured on the axon-tunneled trn2 cores: ~205-235 us device body time
(HBM roofline for the 64 MiB/core input stream at ~358 GB/s is ~187 us);
DVE busy ~142 us is fully hidden. Relative error vs the fp32 jax
reference: ~8.5e-07.
"""

import numpy as np

import os

N = 8192
N_CORES = 8
ROWS = N // N_CORES  # 1024 rows per core
P = 128  # partitions
CHUNK = int(os.environ.get("K_CHUNK", "4096"))
N_CHUNKS = N // CHUNK
N_RTILES = ROWS // P
IO_BUFS = int(os.environ.get("K_IO_BUFS", "3"))
# K_Q3=1: rotate input DMAs over three queues (SP, ACT, SWDGE) instead
# of two, probing whether per-ring dispatch overhead is the residual.
Q3 = os.environ.get("K_Q3", "0") == "1"

_CACHE = {}


def _build(reps=1):
    import concourse.bacc as bacc
    import concourse.mybir as mybir
    from concourse import tile

    f32 = mybir.dt.float32
    Alu = mybir.AluOpType

    nc = bacc.Bacc(
        "TRN2",
        target_bir_lowering=False,
        debug=False,
        num_devices=N_CORES,
    )

    adj = nc.dram_tensor("adj", [ROWS, N], f32, kind="ExternalInput").ap()
    linv = nc.dram_tensor("linv", [ROWS, N], f32, kind="ExternalInput").ap()
    w = nc.dram_tensor("w", [1, N], f32, kind="ExternalInput").ap()
    b = nc.dram_tensor("b", [1, 1], f32, kind="ExternalInput").ap()
    out = nc.dram_tensor("out", [P, N_RTILES], f32, kind="ExternalOutput").ap()

    with tile.TileContext(nc) as tc:
        with (
            tc.tile_pool(name="consts", bufs=1) as consts,
            tc.tile_pool(name="io", bufs=IO_BUFS) as io,
            tc.tile_pool(name="sink", bufs=1) as sink,
            tc.tile_pool(name="small", bufs=2) as small,
        ):
            # W broadcast to all partitions, resident for the whole kernel.
            # ACT HWDGE ring (SWDGE stride-0 broadcast hangs the device).
            wb = consts.tile([P, N], f32)
            nc.scalar.dma_start(out=wb[:], in_=w.broadcast_to([P, N]))
            # b broadcast to all partitions.
            b_bc = consts.tile([P, 1], f32)
            nc.scalar.dma_start(out=b_bc[:], in_=b.broadcast_to([P, 1]))

            for rep in range(reps):
                stage = small.tile([P, N_RTILES], f32, tag="stage")
                for rt in range(N_RTILES):
                    r0 = rt * P
                    acc = small.tile([P, N_CHUNKS], f32, tag="acc")
                    for ch in range(N_CHUNKS):
                        c0 = ch * CHUNK
                        a_t = io.tile([P, CHUNK], f32, tag="a")
                        l_t = io.tile([P, CHUNK], f32, tag="l")
                        if Q3:
                            # Rotate over three DMA queues; a and l of the
                            # same chunk always land on different queues.
                            rings = (nc.sync, nc.scalar, nc.gpsimd)
                            k = rt * N_CHUNKS + ch
                            a_eng = rings[k % 3]
                            l_eng = rings[(k + 1) % 3]
                        else:
                            # Two input streams on the two HWDGE rings.
                            a_eng, l_eng = nc.sync, nc.scalar
                        a_eng.dma_start(
                            out=a_t[:], in_=adj[r0 : r0 + P, c0 : c0 + CHUNK]
                        )
                        l_eng.dma_start(
                            out=l_t[:], in_=linv[r0 : r0 + P, c0 : c0 + CHUNK]
                        )
                        # In-place product: a_t <- a_t * l_t (identical APs
                        # are safe on the DVE streaming pipe).
                        nc.vector.tensor_mul(out=a_t[:], in0=a_t[:], in1=l_t[:])
                        s = sink.tile([P, CHUNK], f32, tag="s")
                        nc.vector.scalar_tensor_tensor(
                            out=s[:],
                            in0=a_t[:],
                            scalar=0.0,
                            in1=wb[:, c0 : c0 + CHUNK],
                            op0=Alu.max,
                            op1=Alu.mult,
                            accum_out=acc[:, ch : ch + 1],
                        )
                    # stage[:, rt] = b + sum(acc)
                    res = small.tile([P, 1], f32, tag="res")
                    nc.vector.tensor_reduce(
                        out=res[:], in_=acc[:], axis=mybir.AxisListType.X, op=Alu.add
                    )
                    nc.vector.tensor_add(
                        out=stage[:, rt : rt + 1], in0=res[:], in1=b_bc[:]
                    )
                nc.sync.dma_start(out=out[:, :], in_=stage[:])

    nc.compile()
    return nc


def get_nc(reps=1):
    key = ("nc", reps)
    if key not in _CACHE:
        _CACHE[key] = _build(reps)
    return _CACHE[key]


def make_in_maps(adjacency, Linv, W, b):
    adjacency = np.ascontiguousarray(adjacency, dtype=np.float32)
    Linv = np.ascontiguousarray(Linv, dtype=np.float32)
    W = np.ascontiguousarray(W, dtype=np.float32).reshape(1, N)
    b = np.ascontiguousarray(b, dtype=np.float32).reshape(1, 1)
    in_maps = []
    for c in range(N_CORES):
        r0, r1 = c * ROWS, (c + 1) * ROWS
        in_maps.append(
            {
                "adj": adjacency[r0:r1],
                "linv": Linv[r0:r1],
                "w": W,
                "b": b,
            }
        )
    return in_maps


def unstage(core_out, b=0.0):
    """Device staging layout -> [1024, 1] output rows for one core.

    [128, 8], element [p, rt] = row rt*128 + p (b already added on
    device).
    """
    return np.ascontiguousarray(core_out.T).reshape(ROWS, 1)


def kernel(x_e=None, Linv=None, adjacency=None, W=None, b=None, **_unused):
    from concourse.bass_utils import run_bass_kernel_spmd

    nc = get_nc()
    in_maps = make_in_maps(adjacency, Linv, W, b)
    res = run_bass_kernel_spmd(nc, in_maps, core_ids=list(range(N_CORES)))
    out = np.concatenate([unstage(r["out"], b) for r in res.results], axis=0)
    return out.astype(np.float32)
